# revision 8
# baseline (speedup 1.0000x reference)
"""CrossCosineEmbeddingLoss kernel for 8 trn2 NeuronCores.

loss = mean over all (i,j) of: 1 - cos(x_i, y_j) if i==j else relu(cos(x_i, y_j))

Identity used:  total = sum_ij relu(sim_ij) + sum_i (1 - sim_ii - relu(sim_ii))
Sharding: rows of x across 8 cores (1024 rows each); y replicated.

Per-core pipeline:
  - x shard: ACT sumsq -> 1/||x|| -> scale -> PE transpose -> bf16 xhatT
  - y (8 groups of 8 tiles): DVE sumsq -> 1/||y|| -> GpSimd in-place scale
    -> PE transpose -> bf16 yT   (норm folded in, so reduces can mix blocks)
  - main: 32 pairs of j-blocks: 4 bf16 matmuls -> [128,2048] PSUM ->
    fused relu+accum on ACT (direct) or DVE, split for engine balance
  - diagonal correction from natural-layout fp32 tiles
Host combines [128,2] partials from each core.
"""

import numpy as np

import concourse.bacc as bacc
import concourse.bass as bass
import concourse.tile as tile
from concourse import mybir
from concourse.bass_utils import run_bass_kernel_spmd
from concourse.masks import make_identity

N, D = 8192, 128
NCORES = 8
SH = N // NCORES          # 1024 rows of x per core
TX = SH // 128            # 8 x-tiles per core
TY = N // 128             # 64 y-tiles
YG = 8                    # y load groups (8 tiles each)
NP = TY // 2              # 32 main pairs

f32 = mybir.dt.float32
bf16 = mybir.dt.bfloat16
AF = mybir.ActivationFunctionType
ALU = mybir.AluOpType
AX = mybir.AxisListType

ACT_PAIRS = 17  # of the 32 main reduces, how many go to ACT


def _reduce_on_act(p):
    return (p * ACT_PAIRS) % NP < ACT_PAIRS


_CACHE = {}


def _build():
    if "nc" in _CACHE:
        return _CACHE["nc"]
    nc = bacc.Bacc("TRN2", target_bir_lowering=False, debug=False,
                   num_devices=NCORES)
    xs_d = nc.dram_tensor("xs", [SH, D], f32, kind="ExternalInput")
    y_d = nc.dram_tensor("y", [N, D], f32, kind="ExternalInput")
    yd_d = nc.dram_tensor("yd", [SH, D], f32, kind="ExternalInput")
    out_d = nc.dram_tensor("out", [128, 2], f32, kind="ExternalOutput")

    with tile.TileContext(nc) as tc:
        with (
            tc.tile_pool(name="singles", bufs=1) as singles,
            tc.tile_pool(name="yload", bufs=3) as yload,
            tc.tile_pool(name="scrA", bufs=2) as scrA,
            tc.tile_pool(name="scrD", bufs=2) as scrD,
        ):
            ident = singles.tile([128, 128], f32)
            make_identity(nc, ident[:])

            yT = singles.tile([128, TY, 128], bf16)     # [d, jt, j] scaled-transposed y
            xhatT = singles.tile([128, TX, 128], bf16)  # [d, it, i]
            xnat = singles.tile([128, TX, 128], f32)
            xhat = singles.tile([128, TX, 128], f32)
            ydn = singles.tile([128, TX, 128], f32)
            nx2 = singles.tile([128, TX], f32)
            rnx = singles.tile([128, TX], f32)
            ny2 = singles.tile([128, TY], f32)
            rny = singles.tile([128, TY], f32)
            t2y = singles.tile([128, TY], f32)
            nyd2 = singles.tile([128, TX], f32)
            rnyd = singles.tile([128, TX], f32)
            Racc = singles.tile([128, NP], f32)         # per-pair relu sums
            d2 = singles.tile([128, TX], f32)
            sim_d = singles.tile([128, TX], f32)
            relu_d = singles.tile([128, TX], f32)
            t1x = singles.tile([128, TX], f32)
            outsb = singles.tile([128, 2], f32)

            # ---- load x shard / diag rows: rows r = 8p + t -> (p, t, d)
            nc.sync.dma_start(
                out=xnat[:], in_=xs_d[:].rearrange("(p t) d -> p t d", t=TX))
            nc.sync.dma_start(
                out=ydn[:], in_=yd_d[:].rearrange("(p t) d -> p t d", t=TX))

            # ---- x norms + scale
            for t in range(TX):
                nc.scalar.activation(
                    scrA.tile([128, 2048], f32, tag='sa', name='sa')[:, :128],
                    xnat[:, t, :], AF.Square, accum_out=nx2[:, t:t + 1])
            nc.vector.reciprocal(t1x[:], nx2[:])
            nc.scalar.sqrt(rnx[:], t1x[:])   # 1/||x_r||
            for t in range(TX):
                nc.scalar.mul(xhat[:, t, :], xnat[:, t, :], rnx[:, t:t + 1])

            with tc.tile_pool(name="tpsum", bufs=2, space="PSUM") as tpsum:
                ptx = tpsum.tile([128, 1024], f32, tag="tp")
                for t in range(TX):
                    nc.tensor.transpose(ptx[:, 128 * t:128 * (t + 1)],
                                        xhat[:, t, :], ident[:])
                nc.vector.tensor_copy(
                    out=xhatT[:].rearrange("p a b -> p (a b)"), in_=ptx[:])

                # ---- y groups: sumsq -> rny -> gpsimd scale -> transpose
                for g in range(YG):
                    yt = yload.tile([128, TX, 128], f32, tag="yt")
                    nc.sync.dma_start(
                        out=yt[:],
                        in_=y_d[1024 * g:1024 * (g + 1), :]
                        .rearrange("(p t) d -> p t d", t=TX))
                    gs = slice(g * TX, (g + 1) * TX)
                    for k in range(TX):
                        col = g * TX + k
                        nc.vector.scalar_tensor_tensor(
                            out=scrD.tile([128, 2048], f32, tag='sd', name='sd')[:, :128],
                            in0=yt[:, k, :], scalar=1.0, in1=yt[:, k, :],
                            op0=ALU.mult, op1=ALU.mult,
                            accum_out=ny2[:, col:col + 1])
                    nc.vector.reciprocal(t2y[:, gs], ny2[:, gs])
                    nc.scalar.sqrt(rny[:, gs], t2y[:, gs])  # 1/||y_j||
                    for k in range(TX):
                        col = g * TX + k
                        nc.gpsimd.tensor_scalar(
                            out=yt[:, k, :], in0=yt[:, k, :],
                            scalar1=rny[:, col:col + 1], scalar2=None,
                            op0=ALU.mult)
                    pt = tpsum.tile([128, 1024], f32, tag="tp")
                    for k in range(TX):
                        nc.tensor.transpose(pt[:, 128 * k:128 * (k + 1)],
                                            yt[:, k, :], ident[:])
                    dst = yT[:, gs, :].rearrange("p a b -> p (a b)")
                    nc.scalar.copy(out=dst, in_=pt[:])

            # ---- main: pairs of j-blocks -> [128,2048] PSUM -> relu+accum
            with tc.tile_pool(name="mpsum", bufs=2, space="PSUM") as mpsum:
                rhs = xhatT[:].rearrange("p a b -> p (a b)")  # [128, 1024]
                for p in range(NP):
                    ps = mpsum.tile([128, 2048], f32, tag="mp")
                    for h in range(2):
                        lhsT = yT[:, 2 * p + h, :]
                        base = 1024 * h
                        nc.tensor.matmul(ps[:, base:base + 512],
                                         lhsT, rhs[:, 0:512])
                        nc.tensor.matmul(ps[:, base + 512:base + 1024],
                                         lhsT, rhs[:, 512:1024])
                    if _reduce_on_act(p):
                        nc.scalar.activation(
                            scrA.tile([128, 2048], f32, tag='sa', name='sa')[:],
                            ps[:], AF.Relu, accum_out=Racc[:, p:p + 1])
                    else:
                        nc.vector.tensor_scalar(
                            out=scrD.tile([128, 2048], f32, tag='sd', name='sd')[:],
                            in0=ps[:], scalar1=0.0, scalar2=None,
                            op0=ALU.max, op1=ALU.add,
                            accum_out=Racc[:, p:p + 1])

            # ---- diagonal: sim_ii for local rows
            prod = scrD.tile([128, 2048], f32, tag='sd', name='sd')
            nc.vector.tensor_mul(prod[:, :1024],
                                 xhat[:].rearrange("p a b -> p (a b)"),
                                 ydn[:].rearrange("p a b -> p (a b)"))
            nc.vector.tensor_reduce(
                out=d2[:], in_=prod[:, :1024].rearrange("p (a b) -> p a b", a=TX),
                axis=AX.X, op=ALU.add)
            for t in range(TX):
                nc.scalar.activation(
                    scrA.tile([128, 2048], f32, tag='sa', name='sa')[:, :128],
                    ydn[:, t, :], AF.Square, accum_out=nyd2[:, t:t + 1])
            nc.vector.reciprocal(t1x[:], nyd2[:])
            nc.scalar.sqrt(rnyd[:], t1x[:])
            nc.vector.tensor_mul(sim_d[:], d2[:], rnyd[:])
            nc.scalar.activation(relu_d[:], sim_d[:], AF.Relu)
            nc.vector.scalar_tensor_tensor(
                out=scrD.tile([128, 2048], f32, tag='sd', name='sd')[:, :TX],
                in0=sim_d[:], scalar=1.0, in1=relu_d[:],
                op0=ALU.mult, op1=ALU.add, accum_out=outsb[:, 1:2])

            # ---- final: total of pair sums (norms already folded in)
            nc.vector.tensor_reduce(out=outsb[:, 0:1], in_=Racc[:],
                                    axis=AX.X, op=ALU.add)
            nc.sync.dma_start(out=out_d[:], in_=outsb[:])

    nc.compile()
    _CACHE["nc"] = nc
    return nc


def _in_maps(x, y):
    maps = []
    for c in range(NCORES):
        sl = slice(SH * c, SH * (c + 1))
        maps.append({"xs": np.ascontiguousarray(x[sl]),
                     "y": y,
                     "yd": np.ascontiguousarray(y[sl])})
    return maps


def _combine(results):
    total = 0.0
    for c in range(NCORES):
        o = results[c]["out"].astype(np.float64)
        total += o[:, 0].sum() - o[:, 1].sum() + SH
    return np.float32(total / (float(N) * float(N)))


def _run(x, y, trace=False):
    nc = _build()
    res = run_bass_kernel_spmd(nc, _in_maps(x, y), list(range(NCORES)),
                               trace=trace)
    return _combine(res.results), res


def kernel(x, y):
    x = np.asarray(x, dtype=np.float32)
    y = np.asarray(y, dtype=np.float32)
    loss, _ = _run(x, y, trace=False)
    return loss


# revision 11
# speedup vs baseline: 2.3609x; 2.3609x over previous
"""CrossCosineEmbeddingLoss kernel for 8 trn2 NeuronCores.

loss = mean over all (i,j) of: 1 - cos(x_i, y_j) if i==j else relu(cos(x_i, y_j))

Identity used:  total = sum_ij relu(sim_ij) + sum_i (1 - sim_ii - relu(sim_ii))
Sharding: rows of x across 8 cores (1024 rows each); y replicated.

Per-core pipeline (v3):
  - x shard: ACT sumsq -> 1/||x|| -> ACT scale -> PE transpose -> DMA copy
    to fp32 xhatT
  - y (8 groups of 8 tiles): DVE sumsq; raw PE transpose -> DMA copy to
    fp32 yT (no scaling: 1/||y_j|| applied to per-block row sums at the end)
  - main: 64 j-blocks: 2 fp32r matmuls -> [128,1024] PSUM -> fused
    relu+accum into R[:, t]; three-way split: ACT direct from PSUM,
    DVE direct, or DMA-stage to SBUF + DVE in 2x mode
  - final: R * rny, reduce; diagonal correction from natural fp32 tiles
Host combines [128,2] partials from each core.
"""

import numpy as np

import concourse.bacc as bacc
import concourse.bass as bass
import concourse.tile as tile
from concourse import mybir
from concourse.bass_utils import run_bass_kernel_spmd
from concourse.masks import make_identity

N, D = 8192, 128
NCORES = 8
SH = N // NCORES          # 1024 rows of x per core
TX = SH // 128            # 8 x-tiles per core
TY = N // 128             # 64 y-tiles
YG = 8                    # y load groups (8 tiles each)

f32 = mybir.dt.float32
f32r = mybir.dt.float32r
AF = mybir.ActivationFunctionType
ALU = mybir.AluOpType
AX = mybir.AxisListType

# main-loop reduce assignment, cycle of 4: ACT, staged-DVE, ACT, DVE-direct
def _reduce_kind(t):
    m = t % 4
    if m in (0, 2):
        return "act"
    if m == 1:
        return "staged"
    return "dve"


_CACHE = {}


def _build():
    if "nc" in _CACHE:
        return _CACHE["nc"]
    nc = bacc.Bacc("TRN2", target_bir_lowering=False, debug=False,
                   num_devices=NCORES)
    xs_d = nc.dram_tensor("xs", [SH, D], f32, kind="ExternalInput")
    y_d = nc.dram_tensor("y", [N, D], f32, kind="ExternalInput")
    yd_d = nc.dram_tensor("yd", [SH, D], f32, kind="ExternalInput")
    out_d = nc.dram_tensor("out", [128, 2], f32, kind="ExternalOutput")

    with tile.TileContext(nc) as tc:
        with (
            tc.tile_pool(name="singles", bufs=1) as singles,
            tc.tile_pool(name="yload", bufs=3) as yload,
            tc.tile_pool(name="scrA", bufs=2) as scrA,
            tc.tile_pool(name="scrD", bufs=2) as scrD,
        ):
            ident = singles.tile([128, 128], f32)
            make_identity(nc, ident[:])

            yT = singles.tile([128, TY, 128], f32r)     # [d, jt, j]
            xhatT = singles.tile([128, TX, 128], f32r)  # [d, it, i]
            xnat = singles.tile([128, TX, 128], f32)
            xhat = singles.tile([128, TX, 128], f32)
            ydn = singles.tile([128, TX, 128], f32)
            nx2 = singles.tile([128, TX], f32)
            rnx = singles.tile([128, TX], f32)
            ny2 = singles.tile([128, TY], f32)
            rny = singles.tile([128, TY], f32)
            t2y = singles.tile([128, TY], f32)
            nyd2 = singles.tile([128, TX], f32)
            rnyd = singles.tile([128, TX], f32)
            R = singles.tile([128, TY], f32)
            Ssc = singles.tile([128, TY], f32)
            d2 = singles.tile([128, TX], f32)
            sim_d = singles.tile([128, TX], f32)
            relu_d = singles.tile([128, TX], f32)
            t1x = singles.tile([128, TX], f32)
            outsb = singles.tile([128, 2], f32)

            # ---- load x shard / diag rows: rows r = 8p + t -> (p, t, d)
            nc.sync.dma_start(
                out=xnat[:], in_=xs_d[:].rearrange("(p t) d -> p t d", t=TX))
            nc.sync.dma_start(
                out=ydn[:], in_=yd_d[:].rearrange("(p t) d -> p t d", t=TX))

            # ---- x norms + scale (ACT)
            for t in range(TX):
                nc.vector.scalar_tensor_tensor(
                    out=scrD.tile([128, 1024], f32, tag='sd', name='sd')[:, :128],
                    in0=xnat[:, t, :], scalar=1.0, in1=xnat[:, t, :],
                    op0=ALU.mult, op1=ALU.mult, accum_out=nx2[:, t:t + 1])
            nc.vector.reciprocal(t1x[:], nx2[:])
            nc.scalar.sqrt(rnx[:], t1x[:])   # 1/||x_r||
            for t in range(TX):
                nc.scalar.mul(xhat[:, t, :], xnat[:, t, :], rnx[:, t:t + 1])

            with tc.tile_pool(name="tpsum", bufs=2, space="PSUM") as tpsum:
                ptx = tpsum.tile([128, 1024], f32, tag="tp")
                for t in range(TX):
                    nc.tensor.transpose(ptx[:, 128 * t:128 * (t + 1)],
                                        xhat[:, t, :], ident[:])
                nc.vector.tensor_copy(
                    out=xhatT[:].rearrange("p a b -> p (a b)"), in_=ptx[:])

                # ---- y groups: DVE sumsq + raw transpose + DMA copy out
                for g in range(YG):
                    yt = yload.tile([128, TX, 128], f32, tag="yt")
                    nc.sync.dma_start(
                        out=yt[:],
                        in_=y_d[1024 * g:1024 * (g + 1), :]
                        .rearrange("(p t) d -> p t d", t=TX))
                    gs = slice(g * TX, (g + 1) * TX)
                    for k in range(TX):
                        col = g * TX + k
                        nc.vector.scalar_tensor_tensor(
                            out=scrD.tile([128, 1024], f32, tag='sd', name='sd')[:, :128],
                            in0=yt[:, k, :], scalar=1.0, in1=yt[:, k, :],
                            op0=ALU.mult, op1=ALU.mult,
                            accum_out=ny2[:, col:col + 1])
                    pt = tpsum.tile([128, 1024], f32, tag="tp")
                    for k in range(TX):
                        nc.tensor.transpose(pt[:, 128 * k:128 * (k + 1)],
                                            yt[:, k, :], ident[:])
                    dst = yT[:, gs, :].rearrange("p a b -> p (a b)")
                    if g % 2 == 0:
                        nc.scalar.copy(out=dst, in_=pt[:])
                    else:
                        nc.vector.tensor_copy(out=dst, in_=pt[:])

            # ---- rny = 1/||y_j||
            nc.vector.reciprocal(t2y[:], ny2[:])
            nc.scalar.sqrt(rny[:], t2y[:])

            # ---- main: per j-block fp32r matmuls + fused relu-accumulate
            with tc.tile_pool(name="mpsum", bufs=3, space="PSUM") as mpsum:
                rhs = xhatT[:].rearrange("p a b -> p (a b)")
                for t in range(TY):
                    ps = mpsum.tile([128, 1024], f32, tag="mp")
                    lhsT = yT[:, t, :]
                    nc.tensor.matmul(ps[:, 0:512], lhsT, rhs[:, 0:512])
                    nc.tensor.matmul(ps[:, 512:1024], lhsT, rhs[:, 512:1024])
                    if t % 2 == 0:
                        nc.scalar.activation(
                            ps[:], ps[:], AF.Relu, accum_out=R[:, t:t + 1])
                    else:
                        nc.vector.tensor_scalar(
                            out=ps[:], in0=ps[:], scalar1=0.0, scalar2=None,
                            op0=ALU.max, op1=ALU.add,
                            accum_out=R[:, t:t + 1])

            # ---- diagonal: sim_ii for local rows
            prod = scrD.tile([128, 1024], f32, tag='sd', name='sd')
            nc.vector.tensor_mul(prod[:],
                                 xhat[:].rearrange("p a b -> p (a b)"),
                                 ydn[:].rearrange("p a b -> p (a b)"))
            nc.vector.tensor_reduce(
                out=d2[:], in_=prod[:].rearrange("p (a b) -> p a b", a=TX),
                axis=AX.X, op=ALU.add)
            for t in range(TX):
                nc.vector.scalar_tensor_tensor(
                    out=scrD.tile([128, 1024], f32, tag='sd', name='sd')[:, :128],
                    in0=ydn[:, t, :], scalar=1.0, in1=ydn[:, t, :],
                    op0=ALU.mult, op1=ALU.mult, accum_out=nyd2[:, t:t + 1])
            nc.vector.reciprocal(t1x[:], nyd2[:])
            nc.scalar.sqrt(rnyd[:], t1x[:])
            nc.vector.tensor_mul(sim_d[:], d2[:], rnyd[:])
            nc.scalar.activation(relu_d[:], sim_d[:], AF.Relu)
            nc.vector.scalar_tensor_tensor(
                out=scrD.tile([128, 1024], f32, tag='sd', name='sd')[:, :TX],
                in0=sim_d[:], scalar=1.0, in1=relu_d[:],
                op0=ALU.mult, op1=ALU.add, accum_out=outsb[:, 1:2])

            # ---- final: scale per-block sums by 1/||y_j|| and total
            nc.vector.tensor_mul(Ssc[:], R[:], rny[:])
            nc.vector.tensor_reduce(out=outsb[:, 0:1], in_=Ssc[:],
                                    axis=AX.X, op=ALU.add)
            nc.sync.dma_start(out=out_d[:], in_=outsb[:])

    nc.compile()
    _CACHE["nc"] = nc
    return nc


def _in_maps(x, y):
    maps = []
    for c in range(NCORES):
        sl = slice(SH * c, SH * (c + 1))
        maps.append({"xs": np.ascontiguousarray(x[sl]),
                     "y": y,
                     "yd": np.ascontiguousarray(y[sl])})
    return maps


def _combine(results):
    total = 0.0
    for c in range(NCORES):
        o = results[c]["out"].astype(np.float64)
        total += o[:, 0].sum() - o[:, 1].sum() + SH
    return np.float32(total / (float(N) * float(N)))


def _run(x, y, trace=False):
    nc = _build()
    res = run_bass_kernel_spmd(nc, _in_maps(x, y), list(range(NCORES)),
                               trace=trace)
    return _combine(res.results), res


def kernel(x, y):
    x = np.asarray(x, dtype=np.float32)
    y = np.asarray(y, dtype=np.float32)
    loss, _ = _run(x, y, trace=False)
    return loss


# revision 12
# speedup vs baseline: 2.4859x; 1.0529x over previous
"""CrossCosineEmbeddingLoss kernel for 8 trn2 NeuronCores.

loss = mean over all (i,j) of: 1 - cos(x_i, y_j) if i==j else relu(cos(x_i, y_j))

Identity used:  total = sum_ij relu(sim_ij) + sum_i (1 - sim_ii - relu(sim_ii))
Sharding: rows of x across 8 cores (1024 rows each); y replicated.

Per-core pipeline (v3):
  - x shard: ACT sumsq -> 1/||x|| -> ACT scale -> PE transpose -> DMA copy
    to fp32 xhatT
  - y (8 groups of 8 tiles): DVE sumsq; raw PE transpose -> DMA copy to
    fp32 yT (no scaling: 1/||y_j|| applied to per-block row sums at the end)
  - main: 64 j-blocks: 2 fp32r matmuls -> [128,1024] PSUM -> fused
    relu+accum into R[:, t]; three-way split: ACT direct from PSUM,
    DVE direct, or DMA-stage to SBUF + DVE in 2x mode
  - final: R * rny, reduce; diagonal correction from natural fp32 tiles
Host combines [128,2] partials from each core.
"""

import numpy as np

import concourse.bacc as bacc
import concourse.bass as bass
import concourse.tile as tile
from concourse import mybir
from concourse.bass_utils import run_bass_kernel_spmd
from concourse.masks import make_identity

N, D = 8192, 128
NCORES = 8
SH = N // NCORES          # 1024 rows of x per core
TX = SH // 128            # 8 x-tiles per core
TY = N // 128             # 64 y-tiles
YG = 8                    # y load groups (8 tiles each)

f32 = mybir.dt.float32
f32r = mybir.dt.float32r
AF = mybir.ActivationFunctionType
ALU = mybir.AluOpType
AX = mybir.AxisListType

# main-loop reduce assignment, cycle of 4: ACT, staged-DVE, ACT, DVE-direct
def _reduce_kind(t):
    m = t % 4
    if m in (0, 2):
        return "act"
    if m == 1:
        return "staged"
    return "dve"


_CACHE = {}


def _build():
    if "nc" in _CACHE:
        return _CACHE["nc"]
    nc = bacc.Bacc("TRN2", target_bir_lowering=False, debug=False,
                   num_devices=NCORES)
    xs_d = nc.dram_tensor("xs", [SH, D], f32, kind="ExternalInput")
    y_d = nc.dram_tensor("y", [N, D], f32, kind="ExternalInput")
    yd_d = nc.dram_tensor("yd", [SH, D], f32, kind="ExternalInput")
    out_d = nc.dram_tensor("out", [128, 2], f32, kind="ExternalOutput")

    with tile.TileContext(nc) as tc:
        with (
            tc.tile_pool(name="singles", bufs=1) as singles,
            tc.tile_pool(name="yload", bufs=3) as yload,
            tc.tile_pool(name="scrA", bufs=2) as scrA,
            tc.tile_pool(name="scrD", bufs=2) as scrD,
        ):
            ident = singles.tile([128, 128], f32)
            make_identity(nc, ident[:])

            yT = singles.tile([128, TY, 128], f32r)     # [d, jt, j]
            xhatT = singles.tile([128, TX, 128], f32r)  # [d, it, i]
            xnat = singles.tile([128, TX, 128], f32)
            xhat = singles.tile([128, TX, 128], f32)
            ydn = singles.tile([128, TX, 128], f32)
            nx2 = singles.tile([128, TX], f32)
            rnx = singles.tile([128, TX], f32)
            ny2 = singles.tile([128, TY], f32)
            rny = singles.tile([128, TY], f32)
            t2y = singles.tile([128, TY], f32)
            nyd2 = singles.tile([128, TX], f32)
            rnyd = singles.tile([128, TX], f32)
            R = singles.tile([128, TY], f32)
            Ssc = singles.tile([128, TY], f32)
            d2 = singles.tile([128, TX], f32)
            sim_d = singles.tile([128, TX], f32)
            relu_d = singles.tile([128, TX], f32)
            t1x = singles.tile([128, TX], f32)
            outsb = singles.tile([128, 2], f32)

            # ---- load x shard / diag rows: rows r = 8p + t -> (p, t, d)
            nc.sync.dma_start(
                out=xnat[:], in_=xs_d[:].rearrange("(p t) d -> p t d", t=TX))
            nc.sync.dma_start(
                out=ydn[:], in_=yd_d[:].rearrange("(p t) d -> p t d", t=TX))

            # ---- x norms + scale (ACT)
            for t in range(TX):
                nc.vector.scalar_tensor_tensor(
                    out=scrD.tile([128, 1024], f32, tag='sd', name='sd')[:, :128],
                    in0=xnat[:, t, :], scalar=1.0, in1=xnat[:, t, :],
                    op0=ALU.mult, op1=ALU.mult, accum_out=nx2[:, t:t + 1])
            nc.vector.reciprocal(t1x[:], nx2[:])
            nc.scalar.sqrt(rnx[:], t1x[:])   # 1/||x_r||
            for t in range(TX):
                nc.scalar.mul(xhat[:, t, :], xnat[:, t, :], rnx[:, t:t + 1])

            with tc.tile_pool(name="tpsum", bufs=2, space="PSUM") as tpsum:
                ptx = tpsum.tile([128, 1024], f32, tag="tp")
                for t in range(TX):
                    nc.tensor.transpose(ptx[:, 128 * t:128 * (t + 1)],
                                        xhat[:, t, :], ident[:])
                nc.vector.tensor_copy(
                    out=xhatT[:].rearrange("p a b -> p (a b)"), in_=ptx[:])

                # ---- y groups: DVE sumsq + raw transpose + DMA copy out
                for g in range(YG):
                    yt = yload.tile([128, TX, 128], f32, tag="yt")
                    nc.sync.dma_start(
                        out=yt[:],
                        in_=y_d[1024 * g:1024 * (g + 1), :]
                        .rearrange("(p t) d -> p t d", t=TX))
                    gs = slice(g * TX, (g + 1) * TX)
                    for k in range(TX):
                        col = g * TX + k
                        nc.vector.scalar_tensor_tensor(
                            out=scrD.tile([128, 1024], f32, tag='sd', name='sd')[:, :128],
                            in0=yt[:, k, :], scalar=1.0, in1=yt[:, k, :],
                            op0=ALU.mult, op1=ALU.mult,
                            accum_out=ny2[:, col:col + 1])
                    pt = tpsum.tile([128, 1024], f32, tag="tp")
                    for k in range(TX):
                        nc.tensor.transpose(pt[:, 128 * k:128 * (k + 1)],
                                            yt[:, k, :], ident[:])
                    dst = yT[:, gs, :].rearrange("p a b -> p (a b)")
                    nc.scalar.copy(out=dst, in_=pt[:])

            # ---- rny = 1/||y_j||
            nc.vector.reciprocal(t2y[:], ny2[:])
            nc.scalar.sqrt(rny[:], t2y[:])

            # ---- main: per j-block fp32r matmuls + fused relu-accumulate
            with tc.tile_pool(name="mpsum", bufs=3, space="PSUM") as mpsum:
                rhs = xhatT[:].rearrange("p a b -> p (a b)")
                for t in range(TY):
                    ps = mpsum.tile([128, 1024], f32, tag="mp")
                    lhsT = yT[:, t, :]
                    nc.tensor.matmul(ps[:, 0:512], lhsT, rhs[:, 0:512])
                    nc.tensor.matmul(ps[:, 512:1024], lhsT, rhs[:, 512:1024])
                    if (t * 34) % TY < 34:
                        nc.scalar.activation(
                            ps[:], ps[:], AF.Relu, accum_out=R[:, t:t + 1])
                    else:
                        nc.vector.tensor_scalar(
                            out=ps[:], in0=ps[:], scalar1=0.0, scalar2=None,
                            op0=ALU.max, op1=ALU.add,
                            accum_out=R[:, t:t + 1])

            # ---- diagonal: sim_ii for local rows
            prod = scrD.tile([128, 1024], f32, tag='sd', name='sd')
            nc.vector.tensor_mul(prod[:],
                                 xhat[:].rearrange("p a b -> p (a b)"),
                                 ydn[:].rearrange("p a b -> p (a b)"))
            nc.vector.tensor_reduce(
                out=d2[:], in_=prod[:].rearrange("p (a b) -> p a b", a=TX),
                axis=AX.X, op=ALU.add)
            for t in range(TX):
                nc.vector.scalar_tensor_tensor(
                    out=scrD.tile([128, 1024], f32, tag='sd', name='sd')[:, :128],
                    in0=ydn[:, t, :], scalar=1.0, in1=ydn[:, t, :],
                    op0=ALU.mult, op1=ALU.mult, accum_out=nyd2[:, t:t + 1])
            nc.vector.reciprocal(t1x[:], nyd2[:])
            nc.scalar.sqrt(rnyd[:], t1x[:])
            nc.vector.tensor_mul(sim_d[:], d2[:], rnyd[:])
            nc.scalar.activation(relu_d[:], sim_d[:], AF.Relu)
            nc.vector.scalar_tensor_tensor(
                out=scrD.tile([128, 1024], f32, tag='sd', name='sd')[:, :TX],
                in0=sim_d[:], scalar=1.0, in1=relu_d[:],
                op0=ALU.mult, op1=ALU.add, accum_out=outsb[:, 1:2])

            # ---- final: scale per-block sums by 1/||y_j|| and total
            nc.vector.tensor_mul(Ssc[:], R[:], rny[:])
            nc.vector.tensor_reduce(out=outsb[:, 0:1], in_=Ssc[:],
                                    axis=AX.X, op=ALU.add)
            nc.sync.dma_start(out=out_d[:], in_=outsb[:])

    nc.compile()
    _CACHE["nc"] = nc
    return nc


def _in_maps(x, y):
    maps = []
    for c in range(NCORES):
        sl = slice(SH * c, SH * (c + 1))
        maps.append({"xs": np.ascontiguousarray(x[sl]),
                     "y": y,
                     "yd": np.ascontiguousarray(y[sl])})
    return maps


def _combine(results):
    total = 0.0
    for c in range(NCORES):
        o = results[c]["out"].astype(np.float64)
        total += o[:, 0].sum() - o[:, 1].sum() + SH
    return np.float32(total / (float(N) * float(N)))


def _run(x, y, trace=False):
    nc = _build()
    res = run_bass_kernel_spmd(nc, _in_maps(x, y), list(range(NCORES)),
                               trace=trace)
    return _combine(res.results), res


def kernel(x, y):
    x = np.asarray(x, dtype=np.float32)
    y = np.asarray(y, dtype=np.float32)
    loss, _ = _run(x, y, trace=False)
    return loss
